# revision 10
# baseline (speedup 1.0000x reference)
"""Trainium2 Bass kernel for nn_MemoryAugmented (scatter_memory).

Computes, for full inputs x:[64,12,883,64], M:[12,64,64]:
    score = softmax(einsum('blnd,tmd->btnm', x, M), axis=-1)
    out   = einsum('btnm,tmd->btnd', score, M)

Distribution: data-parallel over batch across 8 NeuronCores (8 batches
per core); the small memory bank M is replicated.

The device pipeline runs in 16-bit (fp32 PSUM): fp16 on the input side
(x, l-sum tree, mm1 weights, final output) -- fp16 matmuls run at full
PE rate, the DMA bytes halve vs fp32 (the binding resource: ~11 MB in +
~10.5 MB out per core at ~358 GB/s), and fp16's 2^-11 rounding keeps
end-to-end error ~3e-3. Only exp's output uses bf16 (e^~19 overflows
fp16) and the softmax reciprocal stays fp32 (1/sum underflows fp16).

Engine budget per 1024-row supertile (7 per core), from v4 traces:
  DVE     8x (strided norm-multiply 0.95us + reciprocal 0.22us) -- the
          PSUM evacuation floor; PSUM is readable only by DVE/ACT
  gpsimd  the 3-op l-sum tree (same-queue ops need no semaphores)
  ACT     6x Exp [128,1024] + store issue
  PE      6 mm1 (N=1024) + 48 mm2 (N=130) + LDWEIGHTS
  sync    loads; scalar ring: stores

Per-core dataflow (rows r = (b, n) flattened to 7064, padded to 7*1024):
  host     x -> fp16, laid out [tile, p=(l_half, d), l%6, r]
  load     one 1.5 MB DMA per tile (sync ring, 12 KB runs/part)
  tree     l-sum 6->3->2->1 on gpsimd (fp16); the final l_half sum
           folds into mm1's K=128 contraction (weights replicated)
  mm1      6x matmul(mwT_pair fp16 [128,128], xs [128,1024]) -> logits
  exp      6x ACT Exp [128,1024] PSUM -> SBUF bf16
  mm2      per 128-row chunk: exp_chunk^T @ [blockdiag(M) | ones cols]
           -> [rows, (t0 d | t1 d | sums)] PSUM; DVE strided reciprocal
           of the 12 sums + broadcast multiply normalizes into vn fp16
  store    one 1.5 MB DMA per tile (scalar ring, 12 KB runs/part)
"""
import sys

for _p in ("/opt/trn_rl_repo",):
    if _p not in sys.path:
        sys.path.insert(0, _p)

from contextlib import ExitStack

import numpy as np

import concourse.bass as bass
import concourse.bacc as bacc
import concourse.tile as tile
from concourse import mybir
from concourse._compat import with_exitstack
from concourse.bass_utils import run_bass_kernel_spmd

B, L, N, D = 64, 12, 883, 64
T, MNUM = 12, 64
NCORES = 8
BS = B // NCORES          # 8 batches per core
ROWS = BS * N             # 7064 real rows per core
NTILES = 7                # 7 tiles of 1024 rows (7168, zero-padded)
TR = 1024                 # rows per tile
NCH = TR // 128           # 8 chunks per tile
RP = NTILES * TR
F32 = mybir.dt.float32
BF16 = mybir.dt.bfloat16
FP16 = mybir.dt.float16
F16 = np.float16


def build_consts(M):
    """Host-side layout prep (pure data movement) of the memory bank."""
    M = np.asarray(M, dtype=np.float32)
    mw = np.zeros((128, 6 * 128), np.float32)
    mbd = np.zeros((128, 6 * 130), np.float32)
    for tp in range(6):
        t0, t1 = 2 * tp, 2 * tp + 1
        for lh in range(2):
            mw[lh * 64:(lh + 1) * 64, tp * 128 + 0:tp * 128 + 64] = M[t0].T
            mw[lh * 64:(lh + 1) * 64, tp * 128 + 64:tp * 128 + 128] = M[t1].T
        mbd[0:64, tp * 130 + 0:tp * 130 + 64] = M[t0]
        mbd[64:128, tp * 130 + 64:tp * 130 + 128] = M[t1]
        mbd[0:64, tp * 130 + 128] = 1.0
        mbd[64:128, tp * 130 + 129] = 1.0
    return mw.astype(F16), mbd.astype(F16)


@with_exitstack
def kernel_body(ctx: ExitStack, tc: "tile.TileContext", out: bass.AP,
                x: bass.AP, mw: bass.AP, mbd: bass.AP):
    nc = tc.nc
    consts = ctx.enter_context(tc.tile_pool(name="consts", bufs=1))
    work = ctx.enter_context(tc.tile_pool(name="work", bufs=2))
    psum = ctx.enter_context(tc.tile_pool(name="psum", bufs=1, space="PSUM"))

    mw_sb = consts.tile([128, 6 * 128], FP16)
    nc.scalar.dma_start(out=mw_sb[:], in_=mw[:])
    mbd_sb = consts.tile([128, 6 * 130], FP16)
    nc.scalar.dma_start(out=mbd_sb[:], in_=mbd[:])
    zbias = consts.tile([128, 1], F32)
    nc.vector.memset(zbias[:], 0.0)

    for ti in range(NTILES):
        # ---- load + l-sum tree (6 slabs -> 1) on gpsimd, fp16 ----
        xt = work.tile([128, 6 * TR], FP16, tag="xt", bufs=3)
        nc.sync.dma_start(out=xt[:], in_=x[ti])
        xv = xt[:].rearrange("p (l two r) -> p l two r", two=2, r=TR)
        t3 = work.tile([128, 3 * TR], FP16, tag="t3", bufs=2)
        t3v = t3[:].rearrange("p (l r) -> p l r", l=3)
        nc.gpsimd.tensor_add(t3v, xv[:, :, 0], xv[:, :, 1])
        t2 = work.tile([128, TR], FP16, tag="t2", bufs=2)
        nc.gpsimd.tensor_add(t2[:], t3v[:, 0], t3v[:, 1])
        xs = work.tile([128, TR], FP16, tag="xs", bufs=3)
        nc.gpsimd.tensor_add(xs[:], t2[:], t3v[:, 2])

        # ---- mm1 + exp ----
        exps = []
        for tp in range(6):
            ps_log = psum.tile([128, TR], F32, tag="logits", bufs=2)
            for half in range(2):  # matmul N<=512: one-PSUM-bank output limit
                nc.tensor.matmul(ps_log[:, half * 512:(half + 1) * 512],
                                 mw_sb[:, tp * 128:(tp + 1) * 128],
                                 xs[:, half * 512:(half + 1) * 512],
                                 start=True, stop=True)
            ex = work.tile([128, TR], BF16, tag="exp", bufs=12)
            nc.scalar.activation(ex[:], ps_log[:],
                                 mybir.ActivationFunctionType.Exp, bias=zbias[:])
            exps.append(ex)

        # ---- mm2 + normalize per 128-row chunk ----
        vn = work.tile([128, NCH * T * D], FP16, tag="vn", bufs=2)
        for c in range(NCH):
            ps_val = psum.tile([128, 1024], F32, tag="val", bufs=2)
            for tp in range(6):
                off = 512 * (tp // 3) + 130 * (tp % 3)
                nc.tensor.matmul(ps_val[:, off:off + 130],
                                 exps[tp][:, c * 128:(c + 1) * 128],
                                 mbd_sb[:, tp * 130:(tp + 1) * 130],
                                 start=True, stop=True)
            sums_ap = (ps_val[:].rearrange("p (h r) -> p h r", h=2)
                       [:, :, 0:390]
                       .rearrange("p h (a r) -> p h a r", a=3)
                       [:, :, :, 128:130])
            rec = work.tile([128, 12], F32, tag="rec", bufs=6)
            nc.vector.reciprocal(
                rec[:].rearrange("p (h a t) -> p h a t", h=2, a=3), sums_ap)
            in0 = (ps_val[:].rearrange("p (h r) -> p h r", h=2)
                   [:, :, 0:390]
                   .rearrange("p h (a r) -> p h a r", a=3)
                   [:, :, :, 0:128]
                   .rearrange("p h a (t d) -> p h a t d", t=2))
            in1 = (rec[:].rearrange("p (h a t) -> p h a t", h=2, a=3)
                   .unsqueeze(4)
                   .broadcast_to([128, 2, 3, 2, D]))
            outp = (vn[:, c * 768:(c + 1) * 768]
                    .rearrange("p (h a t d) -> p h a t d", h=2, a=3, t=2))
            nc.vector.tensor_mul(outp, in0, in1)
        # one fully-contiguous store per tile on the ACT HWDGE ring
        # (128 descriptors of 12 KB; host unshuffles [ti, p, c, t*d])
        nc.scalar.dma_start(out=out[ti], in_=vn[:])


_NC_CACHE = {}


def build_nc():
    if "nc" in _NC_CACHE:
        return _NC_CACHE["nc"]
    nc = bacc.Bacc("TRN2", target_bir_lowering=False, debug=False,
                   num_devices=NCORES)
    x_ap = nc.dram_tensor("x_sh", [NTILES, 128, 6 * TR], FP16,
                          kind="ExternalInput").ap()
    mw_ap = nc.dram_tensor("mw", [128, 6 * 128], FP16, kind="ExternalInput").ap()
    mbd_ap = nc.dram_tensor("mbd", [128, 6 * 130], FP16, kind="ExternalInput").ap()
    out_ap = nc.dram_tensor("out", [NTILES, 128, NCH * T * D], FP16,
                            kind="ExternalOutput").ap()
    with tile.TileContext(nc) as tc:
        kernel_body(tc, out_ap, x_ap, mw_ap, mbd_ap)
    nc.compile()
    _NC_CACHE["nc"] = nc
    return nc


def make_in_maps(x, M):
    xf = np.asarray(x).astype(F16)
    mw, mbd = build_consts(M)
    maps = []
    for i in range(NCORES):
        xc = xf[i * BS:(i + 1) * BS]                     # (8, 12, 883, 64)
        xc = xc.reshape(BS, 2, 6, N, D)                  # (b, lh, lr, n, d)
        xc = xc.transpose(0, 3, 1, 4, 2)                 # (b, n, lh, d, lr)
        xc = xc.reshape(ROWS, 2, D, 6)
        xp = np.zeros((RP, 2, D, 6), F16)
        xp[:ROWS] = xc
        xp = (xp.reshape(NTILES, TR, 128, 6)
                .transpose(0, 2, 3, 1)                   # (ti, p, lr, r)
                .reshape(NTILES, 128, 6 * TR))
        maps.append({"x_sh": np.ascontiguousarray(xp), "mw": mw, "mbd": mbd})
    return maps


def unshard_out(res):
    outs = []
    for i in range(NCORES):
        o = np.asarray(res[i]["out"]).astype(np.float32)   # [ti, p, c*768]
        o = (o.reshape(NTILES, 128, NCH, T * D)
              .transpose(0, 2, 1, 3)                       # row = ti*TR+c*128+p
              .reshape(RP, T * D))[:ROWS]
        outs.append(o.reshape(BS, N, T, D).transpose(0, 2, 1, 3))
    return np.ascontiguousarray(np.concatenate(outs, axis=0))


def kernel(x, M):
    nc = build_nc()
    in_maps = make_in_maps(x, M)
    res = run_bass_kernel_spmd(nc, in_maps, list(range(NCORES))).results
    return unshard_out(res)


if __name__ == "__main__":
    rng = np.random.default_rng(0)
    x = rng.standard_normal((B, L, N, D), dtype=np.float32)
    M = (rng.standard_normal((T, MNUM, D), dtype=np.float32) * 0.125).astype(np.float32)
    out = kernel(x, M)
    print("out", out.shape, out.dtype, float(np.abs(out).max()))


# revision 11
# speedup vs baseline: 1.0446x; 1.0446x over previous
"""Trainium2 Bass kernel for nn_MemoryAugmented (scatter_memory).

Computes, for full inputs x:[64,12,883,64], M:[12,64,64]:
    score = softmax(einsum('blnd,tmd->btnm', x, M), axis=-1)
    out   = einsum('btnm,tmd->btnd', score, M)

Distribution: data-parallel over batch across 8 NeuronCores (8 batches
per core); the small memory bank M is replicated.

The device pipeline runs in 16-bit (fp32 PSUM): fp16 on the input side
(x, l-sum tree, mm1 weights, final output) -- fp16 matmuls run at full
PE rate, the DMA bytes halve vs fp32 (the binding resource: ~11 MB in +
~10.5 MB out per core at ~358 GB/s), and fp16's 2^-11 rounding keeps
end-to-end error ~3e-3. Only exp's output uses bf16 (e^~19 overflows
fp16) and the softmax reciprocal stays fp32 (1/sum underflows fp16).

Engine budget per 1024-row supertile (7 per core), from v4 traces:
  DVE     8x (strided norm-multiply 0.95us + reciprocal 0.22us) -- the
          PSUM evacuation floor; PSUM is readable only by DVE/ACT
  gpsimd  the 3-op l-sum tree (same-queue ops need no semaphores)
  ACT     6x Exp [128,1024] + store issue
  PE      6 mm1 (N=1024) + 48 mm2 (N=130) + LDWEIGHTS
  sync    loads; scalar ring: stores

Per-core dataflow (rows r = (b, n) flattened to 7064, padded to 7*1024):
  host     x -> fp16, laid out [tile, p=(l_half, d), l%6, r]
  load     one 1.5 MB DMA per tile (sync ring, 12 KB runs/part)
  tree     l-sum 6->3->2->1 on gpsimd (fp16); the final l_half sum
           folds into mm1's K=128 contraction (weights replicated)
  mm1      6x matmul(mwT_pair fp16 [128,128], xs [128,1024]) -> logits
  exp      6x ACT Exp [128,1024] PSUM -> SBUF bf16
  mm2      per 128-row chunk: exp_chunk^T @ [blockdiag(M) | ones cols]
           -> [rows, (t0 d | t1 d | sums)] PSUM; DVE strided reciprocal
           of the 12 sums + broadcast multiply normalizes into vn fp16
  store    one 1.5 MB DMA per tile (scalar ring, 12 KB runs/part)
"""
import sys

for _p in ("/opt/trn_rl_repo",):
    if _p not in sys.path:
        sys.path.insert(0, _p)

from contextlib import ExitStack

import numpy as np

import concourse.bass as bass
import concourse.bacc as bacc
import concourse.tile as tile
from concourse import mybir
from concourse._compat import with_exitstack
from concourse.bass_utils import run_bass_kernel_spmd

B, L, N, D = 64, 12, 883, 64
T, MNUM = 12, 64
NCORES = 8
BS = B // NCORES          # 8 batches per core
ROWS = BS * N             # 7064 real rows per core
NTILES = 14               # 14 tiles of 512 rows (7168, zero-padded)
TR = 512                  # rows per tile
NCH = TR // 128           # 8 chunks per tile
RP = NTILES * TR
F32 = mybir.dt.float32
BF16 = mybir.dt.bfloat16
FP16 = mybir.dt.float16
F16 = np.float16


def build_consts(M):
    """Host-side layout prep (pure data movement) of the memory bank."""
    M = np.asarray(M, dtype=np.float32)
    mw = np.zeros((128, 6 * 128), np.float32)
    mbd = np.zeros((128, 6 * 130), np.float32)
    for tp in range(6):
        t0, t1 = 2 * tp, 2 * tp + 1
        for lh in range(2):
            mw[lh * 64:(lh + 1) * 64, tp * 128 + 0:tp * 128 + 64] = M[t0].T
            mw[lh * 64:(lh + 1) * 64, tp * 128 + 64:tp * 128 + 128] = M[t1].T
        mbd[0:64, tp * 130 + 0:tp * 130 + 64] = M[t0]
        mbd[64:128, tp * 130 + 64:tp * 130 + 128] = M[t1]
        mbd[0:64, tp * 130 + 128] = 1.0
        mbd[64:128, tp * 130 + 129] = 1.0
    return mw.astype(F16), mbd.astype(F16)


@with_exitstack
def kernel_body(ctx: ExitStack, tc: "tile.TileContext", out: bass.AP,
                x: bass.AP, mw: bass.AP, mbd: bass.AP):
    nc = tc.nc
    consts = ctx.enter_context(tc.tile_pool(name="consts", bufs=1))
    work = ctx.enter_context(tc.tile_pool(name="work", bufs=2))
    psum = ctx.enter_context(tc.tile_pool(name="psum", bufs=1, space="PSUM"))

    mw_sb = consts.tile([128, 6 * 128], FP16)
    nc.scalar.dma_start(out=mw_sb[:], in_=mw[:])
    mbd_sb = consts.tile([128, 6 * 130], FP16)
    nc.scalar.dma_start(out=mbd_sb[:], in_=mbd[:])
    zbias = consts.tile([128, 1], F32)
    nc.vector.memset(zbias[:], 0.0)

    for ti in range(NTILES):
        # ---- load + l-sum tree (6 slabs -> 1) on gpsimd, fp16 ----
        xt = work.tile([128, 6 * TR], FP16, tag="xt", bufs=4)
        nc.sync.dma_start(out=xt[:], in_=x[ti])
        xv = xt[:].rearrange("p (l two r) -> p l two r", two=2, r=TR)
        t3 = work.tile([128, 3 * TR], FP16, tag="t3", bufs=3)
        t3v = t3[:].rearrange("p (l r) -> p l r", l=3)
        nc.gpsimd.tensor_add(t3v, xv[:, :, 0], xv[:, :, 1])
        t2 = work.tile([128, TR], FP16, tag="t2", bufs=2)
        nc.gpsimd.tensor_add(t2[:], t3v[:, 0], t3v[:, 1])
        xs = work.tile([128, TR], FP16, tag="xs", bufs=3)
        nc.vector.tensor_add(xs[:], t2[:], t3v[:, 2])

        # ---- mm1 + exp ----
        exps = []
        for pi in range(3):
            ps_log = psum.tile([128, 1024], F32, tag="logits", bufs=2)
            for half in range(2):
                tp = 2 * pi + half
                nc.tensor.matmul(ps_log[:, half * 512:(half + 1) * 512],
                                 mw_sb[:, tp * 128:(tp + 1) * 128],
                                 xs[:], start=True, stop=True)
            ex = work.tile([128, 1024], BF16, tag="exp", bufs=8)
            nc.scalar.activation(ex[:], ps_log[:],
                                 mybir.ActivationFunctionType.Exp, bias=zbias[:])
            exps.append(ex)

        def expv(tp):
            return exps[tp // 2][:, (tp % 2) * 512:(tp % 2 + 1) * 512]

        # ---- mm2 + normalize per 128-row chunk ----
        vn = work.tile([128, NCH * T * D], FP16, tag="vn", bufs=3)
        for c in range(NCH):
            ps_val = psum.tile([128, 1024], F32, tag="val", bufs=2)
            for tp in range(6):
                off = 512 * (tp // 3) + 130 * (tp % 3)
                nc.tensor.matmul(ps_val[:, off:off + 130],
                                 expv(tp)[:, c * 128:(c + 1) * 128],
                                 mbd_sb[:, tp * 130:(tp + 1) * 130],
                                 start=True, stop=True)
            sums_ap = (ps_val[:].rearrange("p (h r) -> p h r", h=2)
                       [:, :, 0:390]
                       .rearrange("p h (a r) -> p h a r", a=3)
                       [:, :, :, 128:130])
            rec = work.tile([128, 12], F32, tag="rec", bufs=6)
            nc.vector.reciprocal(
                rec[:].rearrange("p (h a t) -> p h a t", h=2, a=3), sums_ap)
            in0 = (ps_val[:].rearrange("p (h r) -> p h r", h=2)
                   [:, :, 0:390]
                   .rearrange("p h (a r) -> p h a r", a=3)
                   [:, :, :, 0:128]
                   .rearrange("p h a (t d) -> p h a t d", t=2))
            in1 = (rec[:].rearrange("p (h a t) -> p h a t", h=2, a=3)
                   .unsqueeze(4)
                   .broadcast_to([128, 2, 3, 2, D]))
            outp = (vn[:, c * 768:(c + 1) * 768]
                    .rearrange("p (h a t d) -> p h a t d", h=2, a=3, t=2))
            nc.vector.tensor_mul(outp, in0, in1)
        # one fully-contiguous store per tile on the sync HWDGE ring
        # (128 descriptors of 6 KB; host unshuffles [ti, p, c, t*d])
        nc.sync.dma_start(out=out[ti], in_=vn[:])


_NC_CACHE = {}


def build_nc():
    if "nc" in _NC_CACHE:
        return _NC_CACHE["nc"]
    nc = bacc.Bacc("TRN2", target_bir_lowering=False, debug=False,
                   num_devices=NCORES)
    x_ap = nc.dram_tensor("x_sh", [NTILES, 128, 6 * TR], FP16,
                          kind="ExternalInput").ap()
    mw_ap = nc.dram_tensor("mw", [128, 6 * 128], FP16, kind="ExternalInput").ap()
    mbd_ap = nc.dram_tensor("mbd", [128, 6 * 130], FP16, kind="ExternalInput").ap()
    out_ap = nc.dram_tensor("out", [NTILES, 128, NCH * T * D], FP16,
                            kind="ExternalOutput").ap()
    with tile.TileContext(nc) as tc:
        kernel_body(tc, out_ap, x_ap, mw_ap, mbd_ap)
    nc.compile()
    _NC_CACHE["nc"] = nc
    return nc


def make_in_maps(x, M):
    xf = np.asarray(x).astype(F16)
    mw, mbd = build_consts(M)
    maps = []
    for i in range(NCORES):
        xc = xf[i * BS:(i + 1) * BS]                     # (8, 12, 883, 64)
        xc = xc.reshape(BS, 2, 6, N, D)                  # (b, lh, lr, n, d)
        xc = xc.transpose(0, 3, 1, 4, 2)                 # (b, n, lh, d, lr)
        xc = xc.reshape(ROWS, 2, D, 6)
        xp = np.zeros((RP, 2, D, 6), F16)
        xp[:ROWS] = xc
        xp = (xp.reshape(NTILES, TR, 128, 6)
                .transpose(0, 2, 3, 1)                   # (ti, p, lr, r)
                .reshape(NTILES, 128, 6 * TR))
        maps.append({"x_sh": np.ascontiguousarray(xp), "mw": mw, "mbd": mbd})
    return maps


def unshard_out(res):
    outs = []
    for i in range(NCORES):
        o = np.asarray(res[i]["out"]).astype(np.float32)   # [ti, p, c*768]
        o = (o.reshape(NTILES, 128, NCH, T * D)
              .transpose(0, 2, 1, 3)                       # row = ti*TR+c*128+p
              .reshape(RP, T * D))[:ROWS]
        outs.append(o.reshape(BS, N, T, D).transpose(0, 2, 1, 3))
    return np.ascontiguousarray(np.concatenate(outs, axis=0))


def kernel(x, M):
    nc = build_nc()
    in_maps = make_in_maps(x, M)
    res = run_bass_kernel_spmd(nc, in_maps, list(range(NCORES))).results
    return unshard_out(res)


if __name__ == "__main__":
    rng = np.random.default_rng(0)
    x = rng.standard_normal((B, L, N, D), dtype=np.float32)
    M = (rng.standard_normal((T, MNUM, D), dtype=np.float32) * 0.125).astype(np.float32)
    out = kernel(x, M)
    print("out", out.shape, out.dtype, float(np.abs(out).max()))
